# revision 14
# baseline (speedup 1.0000x reference)
"""ALiBi multi-head self-attention on 8 Trainium2 NeuronCores.

Problem: B=2, L=2048, D=1024, H=16, Dh=64, f32 I/O.
  q = X@Wq.T+bq; k = X@Wk.T+bk; v = X@Wv.T+bv   (per-head split)
  S = q k^T/sqrt(Dh) + mask,  mask[h,i,j] = pos_bias[h, i-j+L-1]
  out = softmax(S) v  -> concat heads -> @Wo.T + bo

Sharding: core c -> batch b=c//4, heads [4r, 4r+4) with r=c%4.
Each core computes its 4 heads' attention over its batch and a partial
output projection; a 4-way ReduceScatter per batch-group reduces the
partial (2048,1024) projections, each core emitting a distinct 512-row
slice of the final output.

On-core layout (S^T formulation; keys live on PSUM partitions so the
P@V matmul needs no transposes):
  qT/kT: (Dh on partitions, L on columns), two 128-partition tensors
         per projection; partitions 0-63 = even head, 64-127 = odd head.
  S^T chunk = kT_chunk.T @ qT  (lhsT=kT (64,128keys), rhs=qT (64,512q)),
         the odd head row-packed at tile_position=(64,0) -- the two
         row-band matmuls execute concurrently in the PE array.
  P = exp(S^T/8) * exp(mask^T)  -- exp on ScalarE straight out of PSUM,
         mask factor is a host-precomputed sliding-window buffer
         Ebig[p, c] = exp(pb[c - p + 127]); chunk j0 / q-offset q0 uses
         columns [q0 - j0 + 1920, +512).
  O^T += V_chunk_aug.T @ P  with V augmented by 64 ones columns: PSUM
         rows 64-127 of each accumulator all receive the softmax
         denominator (free partition-broadcast by the PE itself), so
         the epilogue is just reciprocal + scale on DVE -- no gpsimd
         partition_broadcast, no single-lane row copies.
  Pipeline: only the kT/qT column groups needed by quarter 0 run ahead
         of the attention loop; the remaining projection groups and the
         per-chunk V projections are interleaved into quarter 0's chunk
         loop so ScalarE starts exp'ing ~25us in instead of ~60us.
  Output projection runs a quarter late; its 4-way bf16 ReduceScatter
         (Shared-scratchpad output) lands mid-quarter and the
         DRAM->DRAM copy into the IO tensor is issued a few chunks
         later on the idle sync queue.
  Final f32 cast + output bias happen on host.

No softmax max-subtraction: |S/8| <= ~4 for any plausible input scale
here, far inside exp's f32/bf16 range. Compute dtype bf16 (PSUM f32);
fp8 was measured (numpy sim of the exact quantization points) to break
the 2e-2 gate, so everything stays bf16.
"""

import sys

sys.path.insert(0, "/opt/trn_rl_repo")

import ml_dtypes
import numpy as np

import concourse.bass as bass
import concourse.mybir as mybir
import concourse.tile as tile
from concourse import bacc
from concourse.bass_utils import run_bass_kernel_spmd

B, L, D, H, DH = 2, 2048, 1024, 16, 64
NC = 8
HPC = H // 4  # heads per core = 4
HD = HPC * DH  # head dims per core = 256
EW = L + 2048 - 128  # Ebig width = 3968
LQ = 512  # q columns per attention sweep (quarter)
NQ = L // LQ  # 4
KC = 128  # key chunk
NKC = L // KC  # 16
SCALE = 1.0 / np.sqrt(DH)

F32 = mybir.dt.float32
BF16 = mybir.dt.bfloat16

_graph_cache = {}


def _build(shared_mask: bool):
    nc = bacc.Bacc("TRN2", target_bir_lowering=False, debug=False, num_devices=NC)

    xt_d = nc.declare_dram_parameter("xt", [D, L], BF16, isOutput=False)
    wqt_d = nc.declare_dram_parameter("wqt", [D, HD], BF16, isOutput=False)
    wkt_d = nc.declare_dram_parameter("wkt", [D, HD], BF16, isOutput=False)
    wvt_d = nc.declare_dram_parameter("wvt", [D, HD], BF16, isOutput=False)
    wot_d = nc.declare_dram_parameter("wot", [HD, D], BF16, isOutput=False)
    bqk_d = nc.declare_dram_parameter("bqk", [128, 4], F32, isOutput=False)
    bv_d = nc.declare_dram_parameter("bv", [1, HD], F32, isOutput=False)
    n_ebig = 1 if shared_mask else HPC
    ebig_d = nc.declare_dram_parameter("ebig", [n_ebig, 128, EW], BF16, isOutput=False)
    out_d = nc.declare_dram_parameter("out", [L // 4, D], BF16, isOutput=True)

    partial_dram = [nc.dram_tensor(f"partial_{t}", [512, D], BF16) for t in range(NQ)]
    rs_out = [nc.dram_tensor(f"rs_out_{t}", [128, D], BF16) for t in range(NQ)]

    with tile.TileContext(nc) as tc:
        with (
            tc.tile_pool(name="const", bufs=1) as cp,
            tc.tile_pool(name="work", bufs=3) as wp,
            tc.tile_pool(name="outp", bufs=2) as op,
            tc.tile_pool(name="mm", bufs=2, space="PSUM") as pp,
            tc.tile_pool(name="acc", bufs=4, space="PSUM") as pa,
        ):
            # ---- input DMAs on THREE queues so the PE can start ~2us in:
            # weights stream on the gpsimd queue while X splits across the
            # sync and vector queues (two parallel HBM streams).
            xts, w_sb = [], {}
            for k in range(8):
                t = cp.tile([128, L], BF16, tag=f"xt{k}", name=f"xt{k}")
                eng = nc.sync if k % 2 == 0 else nc.scalar
                eng.dma_start(out=t[:, :], in_=xt_d[k * 128 : (k + 1) * 128, :])
                xts.append(t)
            for nm, dten in (("k", wkt_d), ("q", wqt_d), ("v", wvt_d)):
                for k in range(8):
                    w = cp.tile([128, HD], BF16, tag=f"w{nm}{k}", name=f"w{nm}{k}")
                    nc.gpsimd.dma_start(out=w[:, :], in_=dten[k * 128 : (k + 1) * 128, :])
                    w_sb[nm, k] = w
            bqk_sb = cp.tile([128, 4], F32, tag="bqk")
            nc.gpsimd.dma_start(out=bqk_sb[:, :], in_=bqk_d[:, :])
            ebig_sb = []
            for e in range(n_ebig):
                t = cp.tile([128, EW], BF16, tag=f"ebig{e}")
                nc.gpsimd.dma_start(out=t[:, :], in_=ebig_d[e, :, :])
                ebig_sb.append(t)
            wotp_sb = []
            for p in range(2):
                t = cp.tile([128, D], BF16, tag=f"wotp{p}", name=f"wotp{p}")
                nc.gpsimd.dma_start(out=t[:, :], in_=wot_d[p * 128 : (p + 1) * 128, :])
                wotp_sb.append(t)
            bv_bc = cp.tile([128, HD], F32, tag="bv_bc")
            nc.gpsimd.dma_start(out=bv_bc[:, :], in_=bv_d[0:1, :].broadcast_to((128, HD)))

            def ebig_for(h):
                return ebig_sb[0] if shared_mask else ebig_sb[h]

            # ---- persistent SBUF tensors -----------------------------
            qkt_sb = {
                (nm, pc): cp.tile([128, L], BF16, tag=f"{nm}T{pc}", name=f"{nm}T{pc}")
                for nm in ("q", "k")
                for pc in range(2)
            }
            # V augmented with 64 ones columns -> PV matmul broadcasts the
            # softmax denominator across PSUM rows 64-127 for free
            v_aug = cp.tile([128, NKC, HPC, 128], BF16, tag="v_aug")
            # ones for the denominator columns: memset the WHOLE tile
            # contiguously (a 4-D strided memset silently under-fills);
            # the per-chunk V evacuations overwrite columns 0:DH.
            nc.vector.memset(v_aug[:, :, :, :], 1.0)
            otp_sb = [cp.tile([128, L], BF16, tag=f"otp{p}", name=f"otp{p}") for p in range(2)]

            # ---- work generators -------------------------------------
            def proj_group(nm, pc, n2):
                bcol0 = 0 if nm == "q" else 2
                ps = pp.tile([128, 1024], F32, tag="mm", name=f"pj_{nm}{pc}{n2}")
                for ng in range(2):
                    for k in range(8):
                        nc.tensor.matmul(
                            ps[:, ng * 512 : (ng + 1) * 512],
                            w_sb[nm, k][:, pc * 128 : (pc + 1) * 128],
                            xts[k][:, n2 * 1024 + ng * 512 : n2 * 1024 + (ng + 1) * 512],
                            start=(k == 0),
                            stop=(k == 7),
                        )
                nc.vector.tensor_scalar_add(
                    qkt_sb[nm, pc][:, n2 * 1024 : (n2 + 1) * 1024],
                    ps[:, :],
                    bqk_sb[:, bcol0 + pc : bcol0 + pc + 1],
                )

            def v_chunk(c):
                ps = pp.tile([128, 1024], F32, tag="mm", name=f"vc{c}")
                for k in range(8):
                    nc.tensor.matmul(
                        ps[:, 0:HD],
                        xts[k][:, c * 128 : (c + 1) * 128],
                        w_sb["v", k][:, :],
                        start=(k == 0),
                        stop=(k == 7),
                    )
                nc.vector.tensor_tensor(
                    out=v_aug[:, c, :, 0:DH],
                    in0=ps[:, 0:HD].rearrange("p (h d) -> p h d", h=HPC),
                    in1=bv_bc[:, :].rearrange("p (h d) -> p h d", h=HPC),
                    op=mybir.AluOpType.add,
                )

            def oproj_tile(tt, qi):
                qt = tt * 4 + qi
                out_sb = op.tile([128, D], BF16, tag="out_sb", bufs=4, name="out_sb")
                pso = pp.tile([128, 1024], F32, tag="mm", name="pso")
                for n2 in range(2):
                    for p in range(2):
                        nc.tensor.matmul(
                            pso[:, n2 * 512 : (n2 + 1) * 512],
                            otp_sb[p][:, qt * 128 : (qt + 1) * 128],
                            wotp_sb[p][:, n2 * 512 : (n2 + 1) * 512],
                            start=(p == 0),
                            stop=(p == 1),
                        )
                nc.vector.tensor_copy(out_sb[:, :], pso[:, :])
                nc.sync.dma_start(
                    out=partial_dram[tt][qi * 128 : (qi + 1) * 128, :],
                    in_=out_sb[:, :],
                )

            def emit_rs(tt):
                # collectives cannot write IO tensors; scratch + end copies
                nc.gpsimd.collective_compute(
                    "ReduceScatter",
                    mybir.AluOpType.add,
                    replica_groups=[[0, 1, 2, 3], [4, 5, 6, 7]],
                    ins=[partial_dram[tt][:, :]],
                    outs=[rs_out[tt][:, :]],
                )

            # ---- lead-in: just what quarter 0 needs ------------------
            proj_group("k", 0, 0)
            proj_group("k", 1, 0)
            proj_group("q", 0, 0)
            proj_group("q", 1, 0)
            v_chunk(0)
            v_chunk(1)

            # extra work interleaved into each quarter's chunk loop
            extras = {t: {} for t in range(NQ)}
            extras[0] = {
                0: [lambda: proj_group("k", 0, 1), lambda: v_chunk(2)],
                1: [lambda: proj_group("k", 1, 1), lambda: v_chunk(3)],
                2: [lambda: proj_group("q", 0, 1), lambda: v_chunk(4)],
                3: [lambda: proj_group("q", 1, 1), lambda: v_chunk(5)],
            }
            for j in range(4, 14):
                extras[0][j] = [lambda c=j + 2: v_chunk(c)]
            for t in range(1, NQ):
                extras[t] = {j: [lambda t=t, q=j - 1: oproj_tile(t - 1, q)] for j in range(1, 5)}
                extras[t][5] = [lambda t=t: emit_rs(t - 1)]

            # ---- attention ------------------------------------------
            for t in range(NQ):
                q0 = t * LQ
                ops = {
                    (pair, sub): pa.tile(
                        [128, LQ], F32, tag="acc", bufs=4, name=f"ops_{t}_{pair}_{sub}"
                    )
                    for pair in range(2)
                    for sub in range(2)
                }
                for j in range(NKC):
                    j0 = j * KC
                    off = q0 - j0 + 1920
                    for pair in range(2):
                        ps = pp.tile([128, 2 * LQ], F32, tag="mm", name=f"ps_{pair}")
                        for sub in range(2):
                            pb = slice(64 * sub, 64 * sub + 64)
                            nc.tensor.matmul(
                                ps[:, sub * LQ : (sub + 1) * LQ],
                                qkt_sb["k", pair][pb, j0 : j0 + KC],
                                qkt_sb["q", pair][pb, q0 : q0 + LQ],
                                start=True,
                                stop=True,
                                tile_position=(64 * sub, 0),
                            )
                        p_sb = wp.tile(
                            [128, 2 * LQ], BF16, tag=f"p_sb{pair}", name=f"p_sb{pair}", bufs=5
                        )
                        nc.scalar.activation(
                            p_sb[:, :],
                            ps[:, :],
                            mybir.ActivationFunctionType.Exp,
                            scale=float(SCALE),
                        )
                        for sub in range(2):
                            nc.vector.tensor_tensor(
                                out=p_sb[:, sub * LQ : (sub + 1) * LQ],
                                in0=p_sb[:, sub * LQ : (sub + 1) * LQ],
                                in1=ebig_for(2 * pair + sub)[:, off : off + LQ],
                                op=mybir.AluOpType.mult,
                            )
                        for sub in range(2):
                            nc.tensor.matmul(
                                ops[pair, sub][:, :],
                                v_aug[:, j, 2 * pair + sub, :],
                                p_sb[:, sub * LQ : (sub + 1) * LQ],
                                start=(j == 0),
                                stop=(j == NKC - 1),
                            )
                    for fn in extras[t].get(j, ()):
                        fn()
                # epilogue: rows 64-127 of each accumulator hold the
                # denominator (replicated); reciprocal in place, then one
                # partition-shifted scale into the packed O^T tile.
                for pair in range(2):
                    for sub in range(2):
                        o = ops[pair, sub]
                        # the custom-DVE reciprocal misreads PSUM on HW
                        # (sim-correct, HW-garbage) -- stage through SBUF
                        dns = wp.tile([64, LQ], F32, tag="dns", bufs=2, name="dns")
                        nc.vector.tensor_copy(dns[:, :], o[64:128, :])
                        rcb = wp.tile([64, LQ], F32, tag="rcb", bufs=2, name="rcb")
                        nc.vector.reciprocal_approx_fast(rcb[:, :], dns[:, :])
                        nc.vector.tensor_tensor(
                            out=otp_sb[pair][64 * sub : 64 * sub + 64, q0 : q0 + LQ],
                            in0=o[0:64, :],
                            in1=rcb[:, :],
                            op=mybir.AluOpType.mult,
                        )
            for qi in range(4):
                oproj_tile(NQ - 1, qi)
            emit_rs(NQ - 1)
            for tt in range(NQ):
                nc.sync.dma_start(
                    out=out_d[tt * 128 : (tt + 1) * 128, :], in_=rs_out[tt][:, :]
                )

    nc.compile()
    return nc


def _get_graph(shared_mask: bool):
    key = bool(shared_mask)
    if key not in _graph_cache:
        _graph_cache[key] = _build(key)
    return _graph_cache[key]


def _make_ebig(pb_row: np.ndarray) -> np.ndarray:
    """Ebig[p, c] = exp(pb[c - p + 127]), p in [0,128), c in [0,EW)."""
    idx = (np.arange(EW)[None, :] - np.arange(128)[:, None]) + 127
    return np.exp(pb_row[idx]).astype(ml_dtypes.bfloat16)


def kernel(queries, Wq, bq, Wk, bk, Wv, bv, Wo, bo, pos_bias):
    queries = np.asarray(queries, dtype=np.float32)
    Wq, Wk, Wv, Wo = (np.asarray(w, dtype=np.float32) for w in (Wq, Wk, Wv, Wo))
    bq, bk, bv, bo = (np.asarray(x, dtype=np.float32) for x in (bq, bk, bv, bo))
    pos_bias = np.asarray(pos_bias, dtype=np.float32)

    shared_mask = bool(np.all(pos_bias == pos_bias[0:1]))
    nc = _get_graph(shared_mask)

    xt = [np.ascontiguousarray(queries[b].T).astype(ml_dtypes.bfloat16) for b in range(B)]
    wqt = np.ascontiguousarray(Wq.T).astype(ml_dtypes.bfloat16)
    wkt = np.ascontiguousarray(Wk.T).astype(ml_dtypes.bfloat16)
    wvt = np.ascontiguousarray(Wv.T).astype(ml_dtypes.bfloat16)
    wot = np.ascontiguousarray(Wo.T).astype(ml_dtypes.bfloat16)

    if shared_mask:
        ebig_all = {None: _make_ebig(pos_bias[0])[None]}
    else:
        ebig_all = {h: _make_ebig(pos_bias[h]) for h in range(H)}

    in_maps = []
    for c in range(NC):
        b, r = c // 4, c % 4
        hs = r * HD  # head-dim slice start
        bqs = bq[hs : hs + HD].reshape(2, 128).T
        bks = bk[hs : hs + HD].reshape(2, 128).T
        bqk = np.ascontiguousarray(np.concatenate([bqs, bks], axis=1))
        if shared_mask:
            ebig = ebig_all[None]
        else:
            ebig = np.stack([ebig_all[4 * r + i] for i in range(HPC)])
        in_maps.append(
            {
                "xt": xt[b],
                "wqt": np.ascontiguousarray(wqt[:, hs : hs + HD]),
                "wkt": np.ascontiguousarray(wkt[:, hs : hs + HD]),
                "wvt": np.ascontiguousarray(wvt[:, hs : hs + HD]),
                "wot": np.ascontiguousarray(wot[hs : hs + HD, :]),
                "bqk": bqk,
                "bv": bv[None, hs : hs + HD] * 1.0,
                "ebig": np.ascontiguousarray(ebig),
            }
        )

    res = run_bass_kernel_spmd(nc, in_maps, core_ids=list(range(NC)))
    out = np.empty((B, L, D), dtype=np.float32)
    for c in range(NC):
        b, r = c // 4, c % 4
        o = np.asarray(res.results[c]["out"], dtype=np.float32)
        for t in range(4):
            qt = 4 * t + r
            out[b, qt * 128 : (qt + 1) * 128, :] = o[t * 128 : (t + 1) * 128, :]
    out += bo[None, None, :]
    return out
